# revision 1
# baseline (speedup 1.0000x reference)
"""Trainium2 Bass kernel for NaiveKHopGraphAttention.

Strategy (no collectives):
  - Host (numpy, integer index work only): sort edges by src node, group
    src nodes into 128-node blocks, assign blocks to (core, slot) so the
    per-slot tile counts are identical across all 8 cores (SPMD-uniform),
    pad each slot's edge list to whole 128-edge tiles with dummy edges.
    Precompute per-tile one-hot matrices A[e,n] / AT[n,e] in bf16.
    V columns of the fused KV weight are permuted to hd-major order so the
    attention-weight broadcast multiply is fully packed (DVE 2x mode); the
    inverse permutation is folded into the Wo rows.
  - Device (per core, identical program, different data), all matmul
    operands bf16 (PE 1 cycle/row vs 4 for f32):
      1. KVX = X @ [Wk.T | Wv_perm.T] -> bf16 rows in DRAM (512B/edge
         gather granularity); QX (own nodes) kept SBUF-resident bf16.
      2. Per slot (~16 tiles): ONE batched indirect-DMA gathers all the
         slot's K|V rows (amortizes the ~1us SWDGE fixed cost), one DMA
         each for the A / AT streams. Tiles processed in groups of 8:
         8 q matmuls into one PSUM span, one fused qk multiply, one
         per-head reduce, one exp (ACT), one packed wv multiply, 8
         segment matmuls accumulating [num | den] in PSUM.
      3. Per slot epilogue: normalize, LayerNorm1 (g1 folded into Wo),
         PE transpose, out-projection, LayerNorm2, store bf16 (host
         converts to f32).
  - Dummy edges have all-zero A columns -> contribute nothing.
    Zero-degree nodes: den + 1e-30 guard.
"""

import sys

if "/opt/trn_rl_repo" not in sys.path:
    sys.path.insert(0, "/opt/trn_rl_repo")

import ml_dtypes
import numpy as np

BF16NP = ml_dtypes.bfloat16

import concourse.bacc as bacc
import concourse.bass as bass
import concourse.mybir as mybir
import concourse.tile as tile
from concourse.bass import IndirectOffsetOnAxis
from concourse.bass_utils import run_bass_kernel_spmd

F32 = mybir.dt.float32
BF16 = mybir.dt.bfloat16
I32 = mybir.dt.int32

NCORES = 8
P = 128
EPS = 1e-5
SENTINEL = 1000.0
DEN_GUARD = 1e-30
GB = 4  # tiles per compute group
KB = 8  # node tiles per projection DMA batch
GC = 1  # tiles per indirect-gather instruction


# ----------------------------------------------------------------------------
# Host-side preprocessing
# ----------------------------------------------------------------------------

def _schedule(src, dst, n_nodes):
    n_blocks = -(-n_nodes // P)
    n_blocks = -(-n_blocks // NCORES) * NCORES
    n_pad = n_blocks * P
    slots = n_blocks // NCORES

    order = np.argsort(src, kind="stable")
    src_s = src[order]
    dst_s = dst[order]

    counts = np.bincount(src, minlength=n_pad)
    node_off = np.zeros(n_pad + 1, dtype=np.int64)
    np.cumsum(counts, out=node_off[1:])
    blk_cnt = counts.reshape(n_blocks, P).sum(axis=1)
    tiles_b = np.maximum(1, -(-blk_cnt // P))

    order_b = np.argsort(-tiles_b, kind="stable")
    slot_tiles = np.empty(slots, dtype=np.int64)
    blk_of = np.empty((NCORES, slots), dtype=np.int64)
    for j in range(slots):
        grp = order_b[j * NCORES : (j + 1) * NCORES]
        blk_of[:, j] = grp
        slot_tiles[j] = tiles_b[grp].max()
    T = int(slot_tiles.sum())

    dsti = np.zeros((NCORES, T * P), dtype=np.int32)
    srcbf = np.full((NCORES, T * P), SENTINEL, dtype=np.float32)
    tile_off = np.zeros(slots + 1, dtype=np.int64)
    np.cumsum(slot_tiles, out=tile_off[1:])
    for c in range(NCORES):
        for j in range(slots):
            b = blk_of[c, j]
            e0, e1 = node_off[b * P], node_off[(b + 1) * P]
            ne = e1 - e0
            o = tile_off[j] * P
            dsti[c, o : o + ne] = dst_s[e0:e1]
            srcbf[c, o : o + ne] = (src_s[e0:e1] - b * P).astype(np.float32)

    return {
        "n_pad": n_pad,
        "slots": slots,
        "T": T,
        "slot_tiles": [int(x) for x in slot_tiles],
        "blk_of": blk_of,
        "dsti": dsti,
        "srcbf": srcbf,
    }


def _prep_inputs(X, attn_window, Wq, bq, Wk, bk, Wv, bv, Wo, bo, g1, b1, g2, b2):
    n_nodes, D = X.shape
    src = np.asarray(attn_window[0]).astype(np.int64)
    dst = np.asarray(attn_window[1]).astype(np.int64)
    sch = _schedule(src, dst, n_nodes)
    n_pad, slots, T = sch["n_pad"], sch["slots"], sch["T"]

    Xp = np.zeros((n_pad, D), dtype=np.float32)
    Xp[:n_nodes] = np.asarray(X, np.float32)
    XT = np.ascontiguousarray(Xp.T)

    # LayerNorm1 affine (g1, b1) folded into the out-projection.
    WoT = np.asarray(Wo, np.float32).T
    Wo2T = np.ascontiguousarray(WoT * np.asarray(g1, np.float32)[:, None])
    BO2 = (np.asarray(b1, np.float32) @ WoT + np.asarray(bo, np.float32))[None, :]

    WvT_p = np.asarray(Wv, np.float32).T
    bv_p = np.asarray(bv, np.float32)

    has_bkv = bool(np.any(np.asarray(bk) != 0) or np.any(np.asarray(bv) != 0))
    has_bq = bool(np.any(np.asarray(bq) != 0))
    has_bo2 = bool(np.any(BO2 != 0))
    has_g2 = bool(np.any(np.asarray(g2) != 1))
    has_b2 = bool(np.any(np.asarray(b2) != 0))
    flags = (has_bkv, has_bq, has_bo2, has_g2, has_b2)

    common = {
        "XT": XT.astype(BF16NP),
        "WKVT": np.ascontiguousarray(
            np.concatenate([np.asarray(Wk, np.float32).T, WvT_p], axis=1)
        ).astype(BF16NP),
        "WQT": np.ascontiguousarray(np.asarray(Wq, np.float32).T).astype(BF16NP),
        "WO2T": Wo2T.astype(BF16NP),
        "IDENT": np.eye(P, dtype=np.float32),
    }
    if has_bkv:
        common["BKVR"] = np.broadcast_to(
            np.concatenate([np.asarray(bk, np.float32), bv_p])[None, :],
            (P, 2 * D)).copy()
    if has_bq:
        common["BQR"] = np.broadcast_to(
            np.asarray(bq, np.float32)[None, :], (P, D)).copy()
    if has_bo2:
        common["BO2R"] = np.broadcast_to(BO2, (P, D)).copy()
    if has_g2:
        common["G2R"] = np.broadcast_to(
            np.asarray(g2, np.float32)[None, :], (P, D)).astype(BF16NP).copy()
    if has_b2:
        common["B2R"] = np.broadcast_to(
            np.asarray(b2, np.float32)[None, :], (P, D)).astype(BF16NP).copy()

    iota = np.arange(P, dtype=np.float32)
    in_maps = []
    for c in range(NCORES):
        blocks = sch["blk_of"][c]
        xtq = np.ascontiguousarray(
            Xp[(blocks[:, None] * P + np.arange(P)[None, :]).ravel()].T)
        srcb3 = sch["srcbf"][c].reshape(T, P)            # [T, e]
        a4 = (srcb3[:, :, None] == iota[None, None, :])  # [T, e, n]
        m = dict(common)
        m["XTQ"] = xtq.astype(BF16NP)
        # kvx rows are stored batch-interleaved: for batch B of KB node
        # tiles, node n=(B*KB+cc)*P+p lands at row B*KB*P + p*KB + cc.
        dd = sch["dsti"][c].astype(np.int64)
        ib = dd // P
        rr = (ib // KB) * (KB * P) + (dd % P) * KB + ib % KB
        m["DSTI"] = np.ascontiguousarray(
            rr.astype(np.int32).reshape(T, P).T)
        # A: [e_part, T*n]; AT: [n_part, T*e]
        m["AH"] = np.ascontiguousarray(
            a4.transpose(1, 0, 2).reshape(P, T * P)).astype(BF16NP)
        m["ATH"] = np.ascontiguousarray(
            a4.transpose(2, 0, 1).reshape(P, T * P)).astype(BF16NP)
        in_maps.append(m)
    return sch, in_maps, flags


# ----------------------------------------------------------------------------
# Device kernel
# ----------------------------------------------------------------------------

def _newton_rsqrt(nc, pool, v_ap, tag, iters=2):
    """rstd = 1/sqrt(v) on DVE only (keeps ACT exp-table warm).
    v_ap: [P,1] f32 (variance + eps already added)."""
    y = pool.tile([P, 1], F32, tag=tag + "_y")
    u = pool.tile([P, 1], I32, tag=tag + "_u")
    nc.vector.tensor_scalar(
        out=u[:], in0=v_ap.bitcast(I32), scalar1=1, scalar2=None,
        op0=mybir.AluOpType.arith_shift_right)
    nc.vector.tensor_scalar(
        out=y[:].bitcast(I32), in0=u[:], scalar1=0x5F3759DF, scalar2=-1,
        op0=mybir.AluOpType.subtract, op1=mybir.AluOpType.mult)
    t = pool.tile([P, 1], F32, tag=tag + "_t")
    for _ in range(iters):
        nc.vector.tensor_mul(t[:], y[:], y[:])
        nc.vector.tensor_mul(t[:], t[:], v_ap)
        nc.vector.tensor_scalar(
            out=t[:], in0=t[:], scalar1=-0.5, scalar2=1.5,
            op0=mybir.AluOpType.mult, op1=mybir.AluOpType.add)
        nc.vector.tensor_mul(y[:], y[:], t[:])
    return y


def build_program(n_pad, slots, T, slot_tiles, D=128, H=8,
                  flags=(False, False, False, False, False), debug=False):
    has_bkv, has_bq, has_bo2, has_g2, has_b2 = flags
    HD = D // H
    scale = 1.0 / np.sqrt(HD)
    nkv_tiles = n_pad // P
    st_max = max(slot_tiles)

    nc = bacc.Bacc("TRN2", target_bir_lowering=False, debug=False,
                   num_devices=NCORES)

    xt = nc.dram_tensor("XT", [D, n_pad], BF16, kind="ExternalInput").ap()
    xtq = nc.dram_tensor("XTQ", [D, slots * P], BF16, kind="ExternalInput").ap()
    wkvt = nc.dram_tensor("WKVT", [D, 2 * D], BF16, kind="ExternalInput").ap()
    wqt = nc.dram_tensor("WQT", [D, D], BF16, kind="ExternalInput").ap()
    wo2t = nc.dram_tensor("WO2T", [D, D], BF16, kind="ExternalInput").ap()
    ident_in = nc.dram_tensor("IDENT", [P, P], F32, kind="ExternalInput").ap()
    dsti = nc.dram_tensor("DSTI", [P, T], I32, kind="ExternalInput").ap()
    ah = nc.dram_tensor("AH", [P, T * P], BF16, kind="ExternalInput").ap()
    ath = nc.dram_tensor("ATH", [P, T * P], BF16, kind="ExternalInput").ap()
    if has_bkv:
        bkvr = nc.dram_tensor("BKVR", [P, 2 * D], F32, kind="ExternalInput").ap()
    if has_bq:
        bqr = nc.dram_tensor("BQR", [P, D], F32, kind="ExternalInput").ap()
    if has_bo2:
        bo2r = nc.dram_tensor("BO2R", [P, D], F32, kind="ExternalInput").ap()
    if has_g2:
        g2r = nc.dram_tensor("G2R", [P, D], BF16, kind="ExternalInput").ap()
    if has_b2:
        b2r = nc.dram_tensor("B2R", [P, D], BF16, kind="ExternalInput").ap()
    out = nc.dram_tensor("OUT", [slots * P, D], BF16, kind="ExternalOutput").ap()

    kvx = nc.dram_tensor("KVXs", [n_pad, 2 * D], BF16,
                         kind="ExternalOutput" if debug else "Internal").ap()
    kvxf = kvx.bitcast(F32)
    st_mx = max(slot_tiles)
    if debug:
        dbg_qx = nc.dram_tensor("DBG_QX", [P, slots * D], BF16,
                                kind="ExternalOutput").ap()
        dbg_kvt = nc.dram_tensor("DBG_KVT", [P, st_mx * 2 * D], BF16,
                                 kind="ExternalOutput").ap()
        dbg_sc = nc.dram_tensor("DBG_SC", [P, GB * H], F32,
                                kind="ExternalOutput").ap()
        dbg_rhs = nc.dram_tensor("DBG_RHS", [P, GB * (D + H)], BF16,
                                 kind="ExternalOutput").ap()
        dbg_den = nc.dram_tensor("DBG_DEN", [P, H], F32,
                                 kind="ExternalOutput").ap()
        dbg_attn = nc.dram_tensor("DBG_ATTN", [P, D], BF16,
                                  kind="ExternalOutput").ap()

    with tile.TileContext(nc) as tc:
        with (
            tc.tile_pool(name="consts", bufs=1) as consts,
            tc.tile_pool(name="proj_in", bufs=3) as proj_in,
            tc.tile_pool(name="proj_out", bufs=3) as proj_out,
            tc.tile_pool(name="astream", bufs=3) as astream,
            tc.tile_pool(name="gath", bufs=4) as gath,
            tc.tile_pool(name="edges", bufs=4) as edges,
            tc.tile_pool(name="blk", bufs=2) as blk,
            tc.tile_pool(name="mmq", bufs=3, space="PSUM") as mmq,
            tc.tile_pool(name="mmseg", bufs=2, space="PSUM") as mmseg,
            tc.tile_pool(name="mm1", bufs=1, space="PSUM") as mm1,
        ):
            # ---- constants
            c_wkvt = consts.tile([D, 2 * D], BF16, tag="wkvt")
            nc.sync.dma_start(out=c_wkvt[:], in_=wkvt[:])
            c_wqt = consts.tile([D, D], BF16, tag="wqt")
            nc.sync.dma_start(out=c_wqt[:], in_=wqt[:])
            c_wo2t = consts.tile([D, D], BF16, tag="wo2t")
            nc.sync.dma_start(out=c_wo2t[:], in_=wo2t[:])
            c_ident = consts.tile([P, P], F32, tag="ident")
            nc.sync.dma_start(out=c_ident[:], in_=ident_in[:])
            c_dsti = consts.tile([P, T], I32, tag="dsti")
            nc.sync.dma_start(out=c_dsti[:], in_=dsti[:])
            c_xtq = consts.tile([D, slots * P], BF16, tag="xtq")
            nc.sync.dma_start(out=c_xtq[:], in_=xtq[:])
            if has_bkv:
                c_bkvr = consts.tile([P, 2 * D], F32, tag="bkvr")
                nc.sync.dma_start(out=c_bkvr[:], in_=bkvr[:])
            if has_bq:
                c_bqr = consts.tile([P, D], F32, tag="bqr")
                nc.sync.dma_start(out=c_bqr[:], in_=bqr[:])
            if has_bo2:
                c_bo2r = consts.tile([P, D], F32, tag="bo2r")
                nc.sync.dma_start(out=c_bo2r[:], in_=bo2r[:])
            if has_g2:
                c_g2 = consts.tile([P, D], BF16, tag="g2")
                nc.sync.dma_start(out=c_g2[:], in_=g2r[:])
            if has_b2:
                c_b2 = consts.tile([P, D], BF16, tag="b2")
                nc.sync.dma_start(out=c_b2[:], in_=b2r[:])
            c_qx = consts.tile([P, slots * D], BF16, tag="qx")

            # ---- Q projection (into SBUF-resident c_qx, bf16)
            for j in range(slots):
                ps = mm1.tile([P, D], F32, tag="p1")
                nc.tensor.matmul(out=ps[:], lhsT=c_xtq[:, j * P : (j + 1) * P],
                                 rhs=c_wqt[:], start=True, stop=True)
                if has_bq:
                    nc.vector.tensor_add(c_qx[:, j * D : (j + 1) * D], ps[:],
                                         c_bqr[:])
                else:
                    nc.scalar.copy(c_qx[:, j * D : (j + 1) * D], ps[:])

            # ---- K/V projection for all nodes (interleaved K|Vperm rows)
            kvx_t = kvx.tensor
            for i0 in range(0, nkv_tiles, KB):
                kb = min(KB, nkv_tiles - i0)
                xb = proj_in.tile([P, KB, P], BF16, tag="xin")
                nc.sync.dma_start(
                    out=xb[:, :kb, :],
                    in_=xt[:, i0 * P : (i0 + kb) * P].rearrange(
                        "p (c n) -> p c n", c=kb))
                kvo = proj_out.tile([P, KB, 2 * D], BF16, tag="kvo")
                for c in range(kb):
                    psq = mmq.tile([P, GB, D], F32, tag="qp")
                    ps = psq[:, :2, :].rearrange("p c n -> p (c n)")
                    nc.tensor.matmul(out=ps, lhsT=xb[:, c, :],
                                     rhs=c_wkvt[:], start=True, stop=True)
                    if has_bkv:
                        nc.vector.tensor_add(kvo[:, c, :], ps, c_bkvr[:])
                    elif c % 2 == 0:
                        nc.scalar.copy(kvo[:, c, :], ps)
                    else:
                        nc.vector.tensor_copy(kvo[:, c, :], ps)
                # store kb*P node rows, batch-interleaved: row = p*kb + c
                nc.sync.dma_start(
                    out=kvx[i0 * P : (i0 + kb) * P, :].rearrange(
                        "(p c) n -> p c n", p=P),
                    in_=kvo[:, :kb, :])

            if debug:
                nc.sync.dma_start(out=dbg_qx[:], in_=c_qx[:])

            # ---- edge stage + per-block epilogue
            ti = 0
            for j in range(slots):
                stn = slot_tiles[j]
                qxj = c_qx[:, j * D : (j + 1) * D]

                # gather through an f32 view (rows are 512B either way):
                # 2-dim out APs + 4-byte dtype, matching the proven path.
                kvtf = gath.tile([P, st_max * D], F32, tag="kvt")
                kvts = kvtf[:].bitcast(BF16).rearrange(
                    "p (c n) -> p c n", c=st_max)
                for c0 in range(0, stn, GC):
                    cw = min(GC, stn - c0)
                    nc.gpsimd.indirect_dma_start(
                        out=kvtf[:, c0 * D : (c0 + cw) * D], out_offset=None,
                        in_=kvxf[:],
                        in_offset=IndirectOffsetOnAxis(
                            ap=c_dsti[:, ti + c0 : ti + c0 + cw], axis=0))
                a_s = astream.tile([P, st_max, P], BF16, tag="a")
                nc.sync.dma_start(
                    out=a_s[:, :stn, :],
                    in_=ah[:, ti * P : (ti + stn) * P].rearrange(
                        "p (c n) -> p c n", c=stn))
                at_s = astream.tile([P, st_max, P], BF16, tag="at")
                nc.scalar.dma_start(
                    out=at_s[:, :stn, :],
                    in_=ath[:, ti * P : (ti + stn) * P].rearrange(
                        "p (c n) -> p c n", c=stn))

                ps_seg = mmseg.tile([P, D + H], F32, tag="seg")
                for g0 in range(0, stn, GB):
                    gw = min(GB, stn - g0)
                    qp = mmq.tile([P, GB, D], F32, tag="qp")
                    for c in range(gw):
                        nc.tensor.matmul(out=qp[:, c, :],
                                         lhsT=at_s[:, g0 + c, :], rhs=qxj,
                                         start=True, stop=True)
                    qk = edges.tile([P, GB, D], BF16, tag="qk")
                    nc.vector.tensor_tensor(
                        out=qk[:, :gw, :], in0=qp[:, :gw, :],
                        in1=kvts[:, g0 : g0 + gw, :D],
                        op=mybir.AluOpType.mult)
                    sc = edges.tile([P, GB, H], F32, tag="sc")
                    nc.vector.tensor_reduce(
                        out=sc[:, :gw, :],
                        in_=qk[:, :gw, :].rearrange("p c (h x) -> p c h x", h=H),
                        axis=mybir.AxisListType.X, op=mybir.AluOpType.add)
                    rhs_t = edges.tile([P, GB, D + H], BF16, tag="rhs")
                    nc.scalar.activation(
                        out=rhs_t[:, :gw, D:], in_=sc[:, :gw, :],
                        func=mybir.ActivationFunctionType.Exp, scale=scale)
                    # wv: ex broadcast along head_dim (innermost stride 0)
                    r0 = rhs_t[:]
                    ex_b = bass.AP(
                        r0.tensor, r0.offset + D,
                        [r0.ap[0], [D + H, gw], [1, H], [0, HD]])
                    nc.vector.tensor_tensor(
                        out=rhs_t[:, :gw, :D].rearrange(
                            "p c (h x) -> p c h x", h=H),
                        in0=kvts[:, g0 : g0 + gw, D:].rearrange(
                            "p c (h x) -> p c h x", h=H),
                        in1=ex_b, op=mybir.AluOpType.mult)
                    for c in range(gw):
                        nc.tensor.matmul(out=ps_seg[:], lhsT=a_s[:, g0 + c, :],
                                         rhs=rhs_t[:, c, :],
                                         start=(g0 + c == 0),
                                         stop=(g0 + c == stn - 1))
                    if debug and j == 0 and g0 == 0:
                        nc.sync.dma_start(
                            out=dbg_sc[:],
                            in_=sc[:].rearrange("p c h -> p (c h)"))
                        nc.sync.dma_start(
                            out=dbg_rhs[:],
                            in_=rhs_t[:].rearrange("p c n -> p (c n)"))
                if debug and j == 0:
                    kvt2 = gath.tile([P, st_max * D], F32, tag="kvt2")
                    nc.gpsimd.indirect_dma_start(
                        out=kvt2[:, : stn * D], out_offset=None, in_=kvxf[:],
                        in_offset=IndirectOffsetOnAxis(
                            ap=c_dsti[:, ti : ti + stn], axis=0))
                    nc.sync.dma_start(out=dbg_kvt[:], in_=kvt2[:].bitcast(BF16))
                ti += stn

                # ---- block epilogue
                den = blk.tile([P, H], F32, tag="den")
                nc.vector.tensor_scalar_add(den[:], ps_seg[:, D:], DEN_GUARD)
                rec = blk.tile([P, H], F32, tag="rec")
                nc.vector.reciprocal(rec[:], den[:])
                rec_b = bass.AP(
                    rec[:].tensor, rec[:].offset,
                    [rec[:].ap[0], [1, H], [0, HD]])
                attn = blk.tile([P, D], BF16, tag="attn")
                nc.vector.tensor_tensor(
                    out=attn[:].rearrange("p (h x) -> p h x", h=H),
                    in0=ps_seg[:, :D].rearrange("p (h x) -> p h x", h=H),
                    in1=rec_b, op=mybir.AluOpType.mult)

                if debug and j == 0:
                    nc.sync.dma_start(out=dbg_den[:], in_=den[:])
                    nc.sync.dma_start(out=dbg_attn[:], in_=attn[:])

                # LayerNorm1 (affine folded into WO2T/BO2)
                st = blk.tile([P, 6], F32, tag="st")
                nc.vector.bn_stats(out=st[:], in_=attn[:])
                mv = blk.tile([P, 2], F32, tag="mv")
                nc.vector.bn_aggr(out=mv[:], in_=st[:])
                ve = blk.tile([P, 1], F32, tag="ve")
                nc.vector.tensor_scalar_add(ve[:], mv[:, 1:2], EPS)
                rstd = _newton_rsqrt(nc, blk, ve[:], "r1")
                xh = blk.tile([P, D], F32, tag="xh")
                nc.vector.tensor_scalar(
                    out=xh[:], in0=attn[:], scalar1=mv[:, 0:1],
                    scalar2=rstd[:, 0:1], op0=mybir.AluOpType.subtract,
                    op1=mybir.AluOpType.mult)

                # transpose + out-projection
                pst = mm1.tile([P, D], F32, tag="pst")
                nc.tensor.transpose(out=pst[:], in_=xh[:], identity=c_ident[:])
                lnt = blk.tile([P, D], BF16, tag="lnt")
                nc.scalar.copy(lnt[:], pst[:])
                ps2 = mm1.tile([P, D], F32, tag="p1")
                nc.tensor.matmul(out=ps2[:], lhsT=lnt[:], rhs=c_wo2t[:],
                                 start=True, stop=True)
                o2_ap = ps2[:]
                if has_bo2:
                    o2 = blk.tile([P, D], F32, tag="o2")
                    nc.vector.tensor_add(o2[:], ps2[:], c_bo2r[:])
                    o2_ap = o2[:]

                # LayerNorm2
                st2 = blk.tile([P, 6], F32, tag="st2")
                nc.vector.bn_stats(out=st2[:], in_=o2_ap)
                mv2 = blk.tile([P, 2], F32, tag="mv2")
                nc.vector.bn_aggr(out=mv2[:], in_=st2[:])
                ve2 = blk.tile([P, 1], F32, tag="ve2")
                nc.vector.tensor_scalar_add(ve2[:], mv2[:, 1:2], EPS)
                rstd2 = _newton_rsqrt(nc, blk, ve2[:], "r2")
                fin = blk.tile([P, D], BF16, tag="fin")
                nc.vector.tensor_scalar(
                    out=fin[:], in0=o2_ap, scalar1=mv2[:, 0:1],
                    scalar2=rstd2[:, 0:1], op0=mybir.AluOpType.subtract,
                    op1=mybir.AluOpType.mult)
                fin_ap = fin[:]
                if has_g2:
                    fg = blk.tile([P, D], BF16, tag="fg")
                    nc.vector.tensor_mul(fg[:], fin_ap, c_g2[:])
                    fin_ap = fg[:]
                if has_b2:
                    fb = blk.tile([P, D], BF16, tag="fb")
                    nc.vector.tensor_add(fb[:], fin_ap, c_b2[:])
                    fin_ap = fb[:]
                nc.sync.dma_start(out=out[j * P : (j + 1) * P, :], in_=fin_ap)

    nc.compile()
    return nc


# ----------------------------------------------------------------------------
# Runner / public API
# ----------------------------------------------------------------------------

def _make_runner(nc, n_cores=NCORES):
    """Build a reusable jitted SPMD callable (mirrors bass2jax.run_bass_via_pjrt)."""
    import jax
    from jax.sharding import Mesh, PartitionSpec
    from jax.experimental.shard_map import shard_map
    from concourse import bass2jax

    bass2jax.install_neuronx_cc_hook()
    partition_name = nc.partition_id_tensor.name if nc.partition_id_tensor else None
    in_names, out_names, out_avals, zero_outs = [], [], [], []
    for alloc in nc.m.functions[0].allocations:
        if not isinstance(alloc, mybir.MemoryLocationSet):
            continue
        name = alloc.memorylocations[0].name
        if alloc.kind == "ExternalInput":
            if name != partition_name:
                in_names.append(name)
        elif alloc.kind == "ExternalOutput":
            out_names.append(name)
            shape = tuple(alloc.tensor_shape)
            dtype = mybir.dt.np(alloc.dtype)
            out_avals.append(jax.core.ShapedArray(shape, dtype))
            zero_outs.append(np.zeros(shape, dtype))
    n_params = len(in_names)
    all_names = list(in_names) + list(out_names)
    if partition_name is not None:
        all_names.append(partition_name)

    def _body(*args):
        operands = list(args)
        if partition_name is not None:
            operands.append(bass2jax.partition_id_tensor())
        outs = bass2jax._bass_exec_p.bind(
            *operands, out_avals=tuple(out_avals), in_names=tuple(all_names),
            out_names=tuple(out_names), lowering_input_output_aliases=(),
            sim_require_finite=True, sim_require_nnan=True, nc=nc)
        return tuple(outs)

    devices = jax.devices()[:n_cores]
    mesh = Mesh(np.asarray(devices), ("core",))
    in_specs = (PartitionSpec("core"),) * (n_params + len(out_names))
    out_specs = (PartitionSpec("core"),) * len(out_names)
    fn = jax.jit(shard_map(_body, mesh=mesh, in_specs=in_specs,
                           out_specs=out_specs, check_rep=False),
                 keep_unused=True)
    return fn, mesh, in_names, out_names, out_avals, zero_outs, n_params


_LAST = {}
_CACHE = {}


def _get_program(key, *args):
    if key not in _CACHE:
        _CACHE[key] = build_program(*args)
    return _CACHE[key]


def bench(inputs, iters=10):
    """Time repeated SPMD executions; returns best wall-clock ns per run."""
    import time
    import jax
    from jax.sharding import NamedSharding, PartitionSpec

    if "nc" not in _LAST:
        kernel(**inputs)
    nc, sch, in_maps = _LAST["nc"], _LAST["sch"], _LAST["in_maps"]
    fn, mesh, in_names, out_names, out_avals, zero_outs, n_params = \
        _make_runner(nc)
    shard = NamedSharding(mesh, PartitionSpec("core"))
    concat_in = [
        jax.device_put(
            np.concatenate([np.asarray(in_maps[c][n]) for c in range(NCORES)],
                           axis=0), shard)
        for n in in_names
    ]
    concat_zero = [
        jax.device_put(np.zeros((NCORES * z.shape[0], *z.shape[1:]), z.dtype),
                       shard)
        for z in zero_outs
    ]
    times = []
    for _ in range(iters + 2):
        t0 = time.perf_counter()
        outs = fn(*concat_in, *concat_zero)
        jax.block_until_ready(outs)
        times.append(time.perf_counter() - t0)
    times = sorted(times[2:])
    return times[0] * 1e9


def kernel(X, attn_window, Wq, bq, Wk, bk, Wv, bv, Wo, bo, g1, b1, g2, b2):
    n_nodes, D = X.shape
    H = 8
    sch, in_maps, flags = _prep_inputs(X, attn_window, Wq, bq, Wk, bk, Wv, bv,
                                       Wo, bo, g1, b1, g2, b2)
    key = (sch["n_pad"], sch["slots"], sch["T"], tuple(sch["slot_tiles"]), D,
           flags)
    nc = _get_program(key, sch["n_pad"], sch["slots"], sch["T"],
                      sch["slot_tiles"], D, H, flags)
    _LAST.update(nc=nc, sch=sch, in_maps=in_maps)
    res = run_bass_kernel_spmd(nc, in_maps, core_ids=list(range(NCORES)))
    out = np.empty((n_nodes, D), dtype=np.float32)
    blk_of = sch["blk_of"]
    for c in range(NCORES):
        oc = np.asarray(res.results[c]["OUT"]).astype(np.float32)
        for j in range(sch["slots"]):
            b = int(blk_of[c, j])
            lo = b * P
            hi = min(lo + P, n_nodes)
            if lo < n_nodes:
                out[lo:hi] = oc[j * P : j * P + (hi - lo)]
    return out



# revision 14
# speedup vs baseline: 2.1052x; 2.1052x over previous
"""Trainium2 Bass kernel for NaiveKHopGraphAttention (lane + edge-slab).

Strategy (no collectives, no device-side gather):
  - Host (integer index work only): sort nodes by degree, group into
    128-node blocks of near-equal degree, assign blocks to (core, slot)
    with SPMD-uniform per-slot tile counts. Tile t of a slot holds the
    t-th edge of each of the block's 128 nodes (lane layout). The host
    ships, per core, the transposed X rows of each lane's dst node
    (XET = X[dst].T, bf16) - pure indexing of the input, no float math.
    Pad lanes point at a zero row.
  - Device phases:
      A: Q projection -> SBUF-resident qx (bf16, node-partition).
      C: per slot, per 4-tile group: PE projects K|V per edge
         (lhsT = slab tile, rhs = [Wk.T|Wv.T]) into PSUM; rotating
         engine copies PSUM->SBUF bf16; DVE qk = K*q_bcast; Pool
         per-head score reduce; Act exp (strided into rhs[:, :, D:]);
         DVE wv = V*ex_bcast; PE identity-matmul accumulates each
         tile's [wv | ex] into a per-slot PSUM [num | den] (lane p is
         node p, so the segment sum is a plain copy-accumulate).
         Dummy lanes contribute ex=1, V=0; the denominator is fixed
         exactly with a host-computed dummy count. Then divide and
         store the attn row block into SBUF.
      D: batched epilogue: LayerNorm1 across all slots in wide DVE ops
         (g1 folded into Wo), per-slot PE transpose + out-projection,
         batched LayerNorm2, single output DMA (bf16; host converts).
"""

import sys

if "/opt/trn_rl_repo" not in sys.path:
    sys.path.insert(0, "/opt/trn_rl_repo")

import ml_dtypes
import numpy as np

BF16NP = ml_dtypes.bfloat16

import concourse.bacc as bacc
import concourse.bass as bass
import concourse.mybir as mybir
import concourse.tile as tile
from concourse.bass_utils import run_bass_kernel_spmd

F32 = mybir.dt.float32
BF16 = mybir.dt.bfloat16
I32 = mybir.dt.int32

NCORES = 8
P = 128
EPS = 1e-5
DEN_GUARD = 1e-30
G = 4        # edge tiles per compute group


# ----------------------------------------------------------------------------
# Host-side preprocessing
# ----------------------------------------------------------------------------

def _schedule(src, dst, n_nodes):
    n_blocks = -(-n_nodes // P)
    n_blocks = -(-n_blocks // NCORES) * NCORES
    n_pad = n_blocks * P
    slots = n_blocks // NCORES
    assert n_pad > n_nodes, "need at least one zero pad node"

    deg = np.bincount(src, minlength=n_pad).astype(np.int64)

    perm = np.argsort(-deg, kind="stable")
    pos = np.empty(n_pad, dtype=np.int64)
    pos[perm] = np.arange(n_pad)
    blk = pos // P
    p_of = pos % P
    j_of = blk // NCORES
    c_of = blk % NCORES

    # per-slot tile count = max degree across the slot's 8 blocks
    deg_blk = deg[perm].reshape(n_blocks, P).max(axis=1)
    stn = deg_blk.reshape(slots, NCORES).max(axis=1)
    stn = np.maximum(stn, 1)
    tile_off = np.zeros(slots + 1, dtype=np.int64)
    np.cumsum(stn, out=tile_off[1:])
    T = int(tile_off[-1])

    # lane dst ids: LID[c, p, tile_off[j]+t] = dst of node's t-th edge
    order = np.argsort(src, kind="stable")
    src_s = src[order]
    dst_s = dst[order]
    cnt = np.bincount(src, minlength=n_pad)
    noff = np.zeros(n_pad + 1, dtype=np.int64)
    np.cumsum(cnt, out=noff[1:])
    rank = np.arange(len(order)) - noff[src_s]

    lid = np.full((NCORES, P, T), n_pad - 1, dtype=np.int64)  # pad: zero row
    cs, ps, js = c_of[src_s], p_of[src_s], j_of[src_s]
    lid[cs, ps, tile_off[js] + rank] = dst_s

    dcnt = np.empty((NCORES, P, slots), dtype=np.float32)
    dcnt[c_of, p_of, j_of] = (stn[j_of] - deg).astype(np.float32)

    nodeids = np.empty((NCORES, slots * P), dtype=np.int64)
    nodeids[c_of, j_of * P + p_of] = np.arange(n_pad)

    return {
        "n_pad": n_pad,
        "slots": slots,
        "T": T,
        "slot_tiles": [int(x) for x in stn],
        "lid": lid,
        "dcnt": dcnt,
        "nodeids": nodeids,
    }


def _prep_inputs(X, attn_window, Wq, bq, Wk, bk, Wv, bv, Wo, bo, g1, b1, g2, b2):
    n_nodes, D = X.shape
    src = np.asarray(attn_window[0]).astype(np.int64)
    dst = np.asarray(attn_window[1]).astype(np.int64)
    sch = _schedule(src, dst, n_nodes)
    n_pad, slots, T = sch["n_pad"], sch["slots"], sch["T"]

    Xp = np.zeros((n_pad, D), dtype=np.float32)
    Xp[:n_nodes] = np.asarray(X, np.float32)
    XTb = np.ascontiguousarray(Xp.T).astype(BF16NP)  # [D, n_pad] bf16

    WoT = np.asarray(Wo, np.float32).T
    Wo2T = np.ascontiguousarray(WoT * np.asarray(g1, np.float32)[:, None])
    BO2 = (np.asarray(b1, np.float32) @ WoT + np.asarray(bo, np.float32))[None, :]

    has_bkv = bool(np.any(np.asarray(bk) != 0) or np.any(np.asarray(bv) != 0))
    has_bq = bool(np.any(np.asarray(bq) != 0))
    has_bo2 = bool(np.any(BO2 != 0))
    has_g2 = bool(np.any(np.asarray(g2) != 1))
    has_b2 = bool(np.any(np.asarray(b2) != 0))
    flags = (has_bkv, has_bq, has_bo2, has_g2, has_b2)

    common = {
        "WKVT": np.ascontiguousarray(
            np.concatenate([np.asarray(Wk, np.float32).T,
                            np.asarray(Wv, np.float32).T], axis=1)
        ).astype(BF16NP),
        "WQT": np.ascontiguousarray(np.asarray(Wq, np.float32).T).astype(BF16NP),
        "WO2T": Wo2T.astype(BF16NP),
        "IDENT": np.eye(P, dtype=np.float32).astype(BF16NP),
    }
    if has_bkv:
        common["BKVR"] = np.broadcast_to(
            np.concatenate([np.asarray(bk, np.float32),
                            np.asarray(bv, np.float32)])[None, :],
            (P, 2 * D)).copy()
    if has_bq:
        common["BQR"] = np.broadcast_to(
            np.asarray(bq, np.float32)[None, :], (P, D)).copy()
    if has_bo2:
        common["BO2R"] = np.broadcast_to(BO2, (P, D)).copy()
    if has_g2:
        common["G2R"] = np.broadcast_to(
            np.asarray(g2, np.float32)[None, :], (P, D)).astype(BF16NP).copy()
    if has_b2:
        common["B2R"] = np.broadcast_to(
            np.asarray(b2, np.float32)[None, :], (P, D)).astype(BF16NP).copy()

    in_maps = []
    for c in range(NCORES):
        m = dict(common)
        m["XTQ"] = np.ascontiguousarray(
            Xp[sch["nodeids"][c]].T).astype(BF16NP)
        # edge slab: X rows of each lane's dst, transposed [D, T*P].
        # lid[c] is [P, T] (lane p, tile t); lane order in a tile is p.
        lid_flat = sch["lid"][c].T.ravel()               # t-major, then p
        m["XET"] = np.ascontiguousarray(XTb[:, lid_flat])
        m["DCNT"] = np.ascontiguousarray(sch["dcnt"][c])
        in_maps.append(m)
    return sch, in_maps, flags


# ----------------------------------------------------------------------------
# Device kernel
# ----------------------------------------------------------------------------

def _newton_rsqrt(nc, pool, v_ap, width, tag, iters=2):
    """rstd = 1/sqrt(v) on DVE only. v_ap: [P, width] f32 (eps added)."""
    y = pool.tile([P, width], F32, tag=tag + "_y")
    u = pool.tile([P, width], I32, tag=tag + "_u")
    nc.vector.tensor_scalar(
        out=u[:], in0=v_ap.bitcast(I32), scalar1=1, scalar2=None,
        op0=mybir.AluOpType.arith_shift_right)
    nc.vector.tensor_scalar(
        out=y[:].bitcast(I32), in0=u[:], scalar1=0x5F3759DF, scalar2=-1,
        op0=mybir.AluOpType.subtract, op1=mybir.AluOpType.mult)
    t = pool.tile([P, width], F32, tag=tag + "_t")
    for _ in range(iters):
        nc.vector.tensor_mul(t[:], y[:], y[:])
        nc.vector.tensor_mul(t[:], t[:], v_ap)
        nc.vector.tensor_scalar(
            out=t[:], in0=t[:], scalar1=-0.5, scalar2=1.5,
            op0=mybir.AluOpType.mult, op1=mybir.AluOpType.add)
        nc.vector.tensor_mul(y[:], y[:], t[:])
    return y


def build_program(n_pad, slots, T, slot_tiles, D=128, H=8,
                  flags=(False, False, False, False, False)):
    has_bkv, has_bq, has_bo2, has_g2, has_b2 = flags
    HD = D // H
    scale = 1.0 / np.sqrt(HD)
    S = slots * D

    nc = bacc.Bacc("TRN2", target_bir_lowering=False, debug=False,
                   num_devices=NCORES)

    xet = nc.dram_tensor("XET", [D, T * P], BF16, kind="ExternalInput").ap()
    xtq = nc.dram_tensor("XTQ", [D, slots * P], BF16, kind="ExternalInput").ap()
    wkvt = nc.dram_tensor("WKVT", [D, 2 * D], BF16, kind="ExternalInput").ap()
    wqt = nc.dram_tensor("WQT", [D, D], BF16, kind="ExternalInput").ap()
    wo2t = nc.dram_tensor("WO2T", [D, D], BF16, kind="ExternalInput").ap()
    ident_in = nc.dram_tensor("IDENT", [P, P], BF16, kind="ExternalInput").ap()
    dcnt_in = nc.dram_tensor("DCNT", [P, slots], F32, kind="ExternalInput").ap()
    if has_bkv:
        bkvr = nc.dram_tensor("BKVR", [P, 2 * D], F32, kind="ExternalInput").ap()
    if has_bq:
        bqr = nc.dram_tensor("BQR", [P, D], F32, kind="ExternalInput").ap()
    if has_bo2:
        bo2r = nc.dram_tensor("BO2R", [P, D], F32, kind="ExternalInput").ap()
    if has_g2:
        g2r = nc.dram_tensor("G2R", [P, D], BF16, kind="ExternalInput").ap()
    if has_b2:
        b2r = nc.dram_tensor("B2R", [P, D], BF16, kind="ExternalInput").ap()
    out = nc.dram_tensor("OUT", [slots * P, D], BF16, kind="ExternalOutput").ap()

    with tile.TileContext(nc) as tc:
        with (
            tc.tile_pool(name="consts", bufs=1) as consts,
            tc.tile_pool(name="slab", bufs=3) as slab,
            tc.tile_pool(name="edges", bufs=3) as edges,
            tc.tile_pool(name="blk", bufs=2) as blk,
            tc.tile_pool(name="epi", bufs=1) as epi,
            tc.tile_pool(name="mmb", bufs=2, space="PSUM") as mmb,
            tc.tile_pool(name="mmseg", bufs=2, space="PSUM") as mmseg,
            tc.tile_pool(name="mmd", bufs=1, space="PSUM") as mmd,
        ):
            # ---- constants
            c_wkvt = consts.tile([D, 2 * D], BF16, tag="wkvt")
            nc.sync.dma_start(out=c_wkvt[:], in_=wkvt[:])
            c_wqt = consts.tile([D, D], BF16, tag="wqt")
            nc.sync.dma_start(out=c_wqt[:], in_=wqt[:])
            c_wo2t = consts.tile([D, D], BF16, tag="wo2t")
            nc.sync.dma_start(out=c_wo2t[:], in_=wo2t[:])
            c_ident = consts.tile([P, P], BF16, tag="ident")
            nc.sync.dma_start(out=c_ident[:], in_=ident_in[:])
            c_dcnt = consts.tile([P, slots], F32, tag="dcnt")
            nc.sync.dma_start(out=c_dcnt[:], in_=dcnt_in[:])
            if has_bkv:
                c_bkvr = consts.tile([P, 2 * D], F32, tag="bkvr")
                nc.sync.dma_start(out=c_bkvr[:], in_=bkvr[:])
            if has_bq:
                c_bqr = consts.tile([P, D], F32, tag="bqr")
                nc.sync.dma_start(out=c_bqr[:], in_=bqr[:])
            if has_bo2:
                c_bo2r = consts.tile([P, D], F32, tag="bo2r")
                nc.sync.dma_start(out=c_bo2r[:], in_=bo2r[:])
            if has_g2:
                c_g2 = consts.tile([P, D], BF16, tag="g2")
                nc.sync.dma_start(out=c_g2[:], in_=g2r[:])
            if has_b2:
                c_b2 = consts.tile([P, D], BF16, tag="b2")
                nc.sync.dma_start(out=c_b2[:], in_=b2r[:])
            c_qx = consts.tile([P, slots * D], BF16, tag="qx")
            c_attn = consts.tile([P, slots * D], BF16, tag="attn")

            # ---- Phase A: Q projection
            for j in range(slots):
                xq = slab.tile([D, P], BF16, tag="xq")
                nc.sync.dma_start(out=xq[:], in_=xtq[:, j * P:(j + 1) * P])
                ps = mmb.tile([P, G, 2 * D], F32, tag="pb")
                nc.tensor.matmul(out=ps[:, 0, :D], lhsT=xq[:],
                                 rhs=c_wqt[:], start=True, stop=True)
                if has_bq:
                    nc.vector.tensor_add(c_qx[:, j * D:(j + 1) * D],
                                         ps[:, 0, :D], c_bqr[:])
                else:
                    nc.scalar.copy(c_qx[:, j * D:(j + 1) * D], ps[:, 0, :D])

            # ---- Phase C: edge stage per slot
            ti = 0
            copy_rot = 0
            for j in range(slots):
                stn = slot_tiles[j]
                qxj = c_qx[:, j * D:(j + 1) * D]
                ps_seg = mmseg.tile([P, D + H], F32, tag="seg")

                for g0 in range(0, stn, G):
                    gw = min(G, stn - g0)
                    xe = slab.tile([D, G, P], BF16, tag="xe")
                    nc.sync.dma_start(
                        out=xe[:, :gw, :],
                        in_=xet[:, (ti + g0) * P:(ti + g0 + gw) * P].rearrange(
                            "p (c n) -> p c n", c=gw))
                    kvp = mmb.tile([P, G, 2 * D], F32, tag="pb")
                    for cc in range(gw):
                        nc.tensor.matmul(out=kvp[:, cc, :], lhsT=xe[:, cc, :],
                                         rhs=c_wkvt[:], start=True, stop=True)
                    kv = edges.tile([P, G, 2 * D], BF16, tag="kv")
                    src_ap = kvp[:, :gw, :].rearrange("p c n -> p (c n)")
                    dst_ap = kv[:, :gw, :].rearrange("p c n -> p (c n)")
                    if has_bkv:
                        b0 = c_bkvr[:]
                        b_b = bass.AP(b0.tensor, b0.offset,
                                      [b0.ap[0], [0, gw], [1, 2 * D]])
                        nc.vector.tensor_tensor(
                            out=kv[:, :gw, :], in0=kvp[:, :gw, :],
                            in1=b_b, op=mybir.AluOpType.add)
                    elif copy_rot % 2 == 0:
                        nc.scalar.copy(dst_ap, src_ap)
                    else:
                        nc.vector.tensor_copy(dst_ap, src_ap)
                    copy_rot += 1

                    qk = edges.tile([P, G, D], BF16, tag="qk")
                    q_b = bass.AP(qxj.tensor, qxj.offset,
                                  [qxj.ap[0], [0, gw], [1, D]])
                    nc.vector.tensor_tensor(
                        out=qk[:, :gw, :], in0=kv[:, :gw, :D],
                        in1=q_b, op=mybir.AluOpType.mult)
                    sc = edges.tile([P, G, H], F32, tag="sc")
                    nc.vector.tensor_reduce(
                        out=sc[:, :gw, :],
                        in_=qk[:, :gw, :].rearrange("p c (h x) -> p c h x", h=H),
                        axis=mybir.AxisListType.X, op=mybir.AluOpType.add)
                    rhs = edges.tile([P, G, D + H], BF16, tag="rhs")
                    nc.scalar.activation(
                        out=rhs[:, :gw, D:], in_=sc[:, :gw, :],
                        func=mybir.ActivationFunctionType.Exp, scale=scale)
                    r0 = rhs[:]
                    ex_b = bass.AP(r0.tensor, r0.offset + D,
                                   [r0.ap[0], [D + H, gw], [1, H], [0, HD]])
                    nc.vector.tensor_tensor(
                        out=rhs[:, :gw, :D].rearrange("p c (h x) -> p c h x",
                                                      h=H),
                        in0=kv[:, :gw, D:].rearrange("p c (h x) -> p c h x",
                                                     h=H),
                        in1=ex_b, op=mybir.AluOpType.mult)
                    for cc in range(gw):
                        nc.tensor.matmul(out=ps_seg[:], lhsT=c_ident[:],
                                         rhs=rhs[:, cc, :],
                                         start=(g0 + cc == 0),
                                         stop=(g0 + cc == stn - 1))
                ti += stn

                # ---- slot epilogue
                den = blk.tile([P, H], F32, tag="den")
                nc.vector.tensor_scalar(
                    out=den[:], in0=ps_seg[:, D:], scalar1=c_dcnt[:, j:j + 1],
                    scalar2=DEN_GUARD, op0=mybir.AluOpType.subtract,
                    op1=mybir.AluOpType.add)
                rec = blk.tile([P, H], F32, tag="rec")
                nc.vector.reciprocal(rec[:], den[:])
                rr = rec[:]
                rec_b = bass.AP(rr.tensor, rr.offset,
                                [rr.ap[0], [1, H], [0, HD]])
                nc.vector.tensor_tensor(
                    out=c_attn[:, j * D:(j + 1) * D].rearrange(
                        "p (h x) -> p h x", h=H),
                    in0=ps_seg[:, :D].rearrange("p (h x) -> p h x", h=H),
                    in1=rec_b, op=mybir.AluOpType.mult)

            # ---- Phase D: batched epilogue
            att3 = c_attn[:].rearrange("p (j d) -> p j d", j=slots)

            def layer_norm_wide(x3, x_flat, out_flat, tagp="ln"):
                s1 = epi.tile([P, slots], F32, tag=tagp + "s1")
                nc.vector.tensor_reduce(out=s1[:], in_=x3,
                                        axis=mybir.AxisListType.X,
                                        op=mybir.AluOpType.add)
                nm = epi.tile([P, slots], F32, tag=tagp + "nm")
                nc.vector.tensor_scalar(
                    out=nm[:], in0=s1[:], scalar1=-1.0 / D, scalar2=None,
                    op0=mybir.AluOpType.mult)
                sq = epi.tile([P, S], BF16, tag=tagp + "sq")
                nc.vector.tensor_tensor(out=sq[:], in0=x_flat, in1=x_flat,
                                        op=mybir.AluOpType.mult)
                s2 = epi.tile([P, slots], F32, tag=tagp + "s2")
                nc.vector.tensor_reduce(
                    out=s2[:], in_=sq[:].rearrange("p (j d) -> p j d", j=slots),
                    axis=mybir.AxisListType.X, op=mybir.AluOpType.add)
                ve = epi.tile([P, slots], F32, tag=tagp + "ve")
                nc.vector.tensor_mul(ve[:], nm[:], nm[:])
                nc.vector.scalar_tensor_tensor(
                    out=ve[:], in0=s2[:], scalar=1.0 / D, in1=ve[:],
                    op0=mybir.AluOpType.mult, op1=mybir.AluOpType.subtract)
                nc.vector.tensor_scalar(
                    out=ve[:], in0=ve[:], scalar1=EPS, scalar2=None,
                    op0=mybir.AluOpType.add)
                rstd = _newton_rsqrt(nc, epi, ve[:], slots, tagp + "r")
                nm0, rs0 = nm[:], rstd[:]
                nm_b = bass.AP(nm0.tensor, nm0.offset,
                               [nm0.ap[0], [1, slots], [0, D]])
                rs_b = bass.AP(rs0.tensor, rs0.offset,
                               [rs0.ap[0], [1, slots], [0, D]])
                xm = epi.tile([P, S], BF16, tag=tagp + "xm")
                nc.vector.tensor_tensor(
                    out=xm[:].rearrange("p (j d) -> p j d", j=slots),
                    in0=x3, in1=nm_b, op=mybir.AluOpType.add)
                nc.vector.tensor_tensor(
                    out=out_flat.rearrange("p (j d) -> p j d", j=slots),
                    in0=xm[:].rearrange("p (j d) -> p j d", j=slots),
                    in1=rs_b, op=mybir.AluOpType.mult)

            xh = epi.tile([P, S], BF16, tag="xh")
            layer_norm_wide(att3, c_attn[:], xh[:])

            o2 = epi.tile([P, S], BF16, tag="o2")
            for j in range(slots):
                pst = mmd.tile([P, D], BF16, tag="pd")
                nc.tensor.transpose(out=pst[:], in_=xh[:, j * D:(j + 1) * D],
                                    identity=c_ident[:])
                lnt = blk.tile([P, D], BF16, tag="lnt")
                nc.scalar.copy(lnt[:], pst[:])
                ps2 = mmd.tile([P, D], F32, tag="pd2")
                nc.tensor.matmul(out=ps2[:], lhsT=lnt[:], rhs=c_wo2t[:],
                                 start=True, stop=True)
                if has_bo2:
                    nc.vector.tensor_add(o2[:, j * D:(j + 1) * D], ps2[:],
                                         c_bo2r[:])
                elif j % 2 == 0:
                    nc.scalar.copy(o2[:, j * D:(j + 1) * D], ps2[:])
                else:
                    nc.vector.tensor_copy(o2[:, j * D:(j + 1) * D], ps2[:])

            fin = epi.tile([P, S], BF16, tag="xh")  # reuse xh's buffer
            layer_norm_wide(o2[:].rearrange("p (j d) -> p j d", j=slots),
                            o2[:], fin[:])
            fin_ap = fin[:]
            if has_g2:
                gg = c_g2[:]
                g_b = bass.AP(gg.tensor, gg.offset,
                              [gg.ap[0], [0, slots], [1, D]])
                fg = epi.tile([P, S], BF16, tag="sq")
                nc.vector.tensor_tensor(
                    out=fg[:].rearrange("p (j d) -> p j d", j=slots),
                    in0=fin_ap.rearrange("p (j d) -> p j d", j=slots),
                    in1=g_b, op=mybir.AluOpType.mult)
                fin_ap = fg[:]
            if has_b2:
                bb = c_b2[:]
                b_b = bass.AP(bb.tensor, bb.offset,
                              [bb.ap[0], [0, slots], [1, D]])
                fb = epi.tile([P, S], BF16, tag="xm")
                nc.vector.tensor_tensor(
                    out=fb[:].rearrange("p (j d) -> p j d", j=slots),
                    in0=fin_ap.rearrange("p (j d) -> p j d", j=slots),
                    in1=b_b, op=mybir.AluOpType.add)
                fin_ap = fb[:]
            nc.sync.dma_start(
                out=out[:].rearrange("(j p) d -> p j d", p=P),
                in_=fin_ap.rearrange("p (j d) -> p j d", j=slots))

    nc.compile()
    return nc


# ----------------------------------------------------------------------------
# Runner / public API
# ----------------------------------------------------------------------------

_LAST = {}
_CACHE = {}


def _get_program(key, *args):
    if key not in _CACHE:
        _CACHE[key] = build_program(*args)
    return _CACHE[key]


def kernel(X, attn_window, Wq, bq, Wk, bk, Wv, bv, Wo, bo, g1, b1, g2, b2):
    n_nodes, D = X.shape
    H = 8
    sch, in_maps, flags = _prep_inputs(X, attn_window, Wq, bq, Wk, bk, Wv, bv,
                                       Wo, bo, g1, b1, g2, b2)
    key = (sch["n_pad"], sch["slots"], sch["T"], tuple(sch["slot_tiles"]),
           D, flags)
    nc = _get_program(key, sch["n_pad"], sch["slots"], sch["T"],
                      sch["slot_tiles"], D, H, flags)
    _LAST.update(nc=nc, sch=sch, in_maps=in_maps)
    res = run_bass_kernel_spmd(nc, in_maps, core_ids=list(range(NCORES)))
    out = np.empty((n_nodes, D), dtype=np.float32)
    for c in range(NCORES):
        oc = np.asarray(res.results[c]["OUT"]).astype(np.float32)
        ids = sch["nodeids"][c]
        valid = ids < n_nodes
        out[ids[valid]] = oc[valid]
    return out
